# revision 25
# baseline (speedup 1.0000x reference)
"""SMEAR MoE layer (nn_MoELayer_SMEAR) Trainium2 Bass kernel, v2.

Problem: B=8, L=2048, D=1024, H=4096, E=8, fp32 in/out.
  logits = x @ router_w.T + router_b; probs = softmax(logits) * mask
  up = probs.sum(L) / clip(mask.sum(L), 1)            # [B, E]
  mW1 = up @ W1 ; mW2 = up @ W2 ; mb1 = up @ b1 ; mb2 = up @ b2
  out = relu(x @ mW1.T + mb1) @ mW2.T + mb2

Sharding (8 cores): dp=2 over B x tp=4 over H; host sums the 4 partial
outputs per dp-group.

v2 design (vs v1 which ran merge on PE and serialized phases):
- fp16 weight path end to end (x, W, merged W, hid, out partials).
  Numerically validated: max rel err ~6e-3 vs the 2e-2 budget (bf16 was
  1.7e-2+, too close).
- Weight merge runs on DVE + Pool(gpsimd), NOT on PE, overlapped with
  the MLP. W1 is merged in two H-half passes so L1 can start after the
  first pass; W2 merges under L1's shadow, in two D-half passes so early
  L2 output tiles unblock sooner.
- Merged weights round-trip DRAM in fp16, split into per-batch/per-half
  tensors so Tile's per-tensor DRAM dep tracking gives fine-grained
  readiness.
- MLP keeps each stationary tile for 4 back-to-back matmuls into 4 PSUM
  banks (LDWEIGHTS amortized; 8 banks double-buffer across groups).
- PE order L1(0) L1(1) L2(0) L1(2) L2(1) L1(3) L2(2) L2(3) so W2-merge
  latency hides while keeping only 2 hid buffers resident.
"""

import numpy as np

import concourse.bass as bass
import concourse.bacc as bacc
import concourse.mybir as mybir
import concourse.tile as tile
from concourse.bass_utils import run_bass_kernel_spmd
from concourse.masks import make_identity

P = 128
B, L, D, H, E = 8, 2048, 1024, 4096, 8
NB = 4          # batches per core
HS = H // 4     # h-shard width per core
DS = D // P     # 8 d-subtiles
HSUB = HS // P  # 8 h-subtiles in shard
OSUB = D // P   # 8 output subtiles
TCH = 512       # moving-dim chunk for matmuls
TC = L // TCH   # 4 chunks per batch
HHALF = HS // 2  # merge half-pass width

F32 = mybir.dt.float32
F16 = mybir.dt.float16
AF = mybir.ActivationFunctionType
ALU = mybir.AluOpType
AX = mybir.AxisListType

KMRG = 16           # weight rows merged per matmul (16 rows x 8 experts = 128)
NG = D * HS // (KMRG * HS)  # 64 row-groups per weight matrix

_CACHED_NC = None


def _build():
    nc = bacc.Bacc("TRN2", target_bir_lowering=False, debug=False)

    xT = nc.dram_tensor("xT", [NB, D, L], F16, kind="ExternalInput")
    maskg = nc.dram_tensor("maskg", [P, L // P, NB], F32, kind="ExternalInput")
    rwg = nc.dram_tensor("rwg", [P, DS, E], F16, kind="ExternalInput")
    rb = nc.dram_tensor("rb", [E, 1], F32, kind="ExternalInput")
    # raw weights pre-grouped on host: [g, k*E+e, c] = WT[e, g*KMRG+k, c]
    W1G = nc.dram_tensor("W1G", [D // KMRG, P, HS], F16, kind="ExternalInput")
    W2G = nc.dram_tensor("W2G", [HS // KMRG, P, D], F16, kind="ExternalInput")
    b1g = nc.dram_tensor("b1g", [P, HSUB, E], F32, kind="ExternalInput")
    b2g = nc.dram_tensor("b2g", [P, OSUB, E], F32, kind="ExternalInput")
    ownc = nc.dram_tensor("ownc", [NB, 1], F32, kind="ExternalInput")
    outp = nc.dram_tensor("outp", [NB, D, L], F16, kind="ExternalOutput")

    # merged weights in (superblock, group, batch, row, col) layout so
    # merge writes are 2 big DMAs per superblock and stationary-tile
    # reads are one DMA each
    mW1d = nc.dram_tensor("mW1d", [8, 8, KMRG, NB, HS], F16)
    mW2d = nc.dram_tensor("mW2d", [8, 8, KMRG, NB, D], F16)

    with tile.TileContext(nc) as tc:
        with tc.tile_pool(name="const", bufs=1) as const:
            ident = const.tile([P, P], F32)
            make_identity(nc, ident)
            ones_col = const.tile([P, 1], F32)
            nc.gpsimd.memset(ones_col[:], 1.0)
            ones_row = const.tile([1, P], F32)
            nc.gpsimd.memset(ones_row[:], 1.0)

            rwT_sb = const.tile([P, DS, E], F16)
            nc.sync.dma_start(rwT_sb[:], rwg.ap())
            rb_sb = const.tile([E, 1], F32)
            nc.sync.dma_start(rb_sb[:], rb.ap())
            maskT_sb = const.tile([P, L // P, NB], F32)
            nc.sync.dma_start(maskT_sb[:], maskg.ap())
            b1T_sb = const.tile([P, HSUB, E], F32)
            nc.scalar.dma_start(b1T_sb[:], b1g.ap())
            b2T_sb = const.tile([P, OSUB, E], F32)
            nc.scalar.dma_start(b2T_sb[:], b2g.ap())
            own_sb = const.tile([NB, 1], F32)
            nc.sync.dma_start(own_sb[:], ownc.ap())

            up_sb = const.tile([E, NB], F32)
            upT_sb = const.tile([NB, E], F32)
            upTo_sb = const.tile([NB, E], F32)
            up_bc = const.tile([P, NB, E], F32)
            upo_bc = const.tile([P, NB, E], F32)
            mb1_sb = const.tile([P, NB, HSUB], F32)
            mb2_sb = const.tile([P, NB, OSUB], F32)
            invbc_sb = const.tile([P, NB], F32)
            # block-diag merge stationary: upblk[k*E+e, k*NB+b] = up[b, e]
            upblk = const.tile([P, NB * KMRG], F16)
            nc.gpsimd.memset(upblk[:], 0.0)

            # rwp/xp open before the router so the W1 merge chunks and
            # first x batches stream during phase B.
            rwp = tc.alloc_tile_pool(name="rwp", bufs=4)
            xp = tc.alloc_tile_pool(name="xp", bufs=2)

            def load_rw4(rawv, g4):
                # one 1MB DMA covering 4 row-groups; big transfers fan out
                # across DMA engines, small ones don't
                rw = rwp.tile([P, 4, HS], F16, tag="rw", name="rw")
                eng = nc.sync if g4 % 2 else nc.scalar
                eng.dma_start(
                    rw[:], rawv[4 * g4:4 * g4 + 4].rearrange("g p h -> p g h"))
                return rw

            def load_x(b):
                xb = xp.tile([P, DS, L], F16, tag="x", name="xb")
                nc.gpsimd.dma_start(
                    xb[:], xT.ap()[b].rearrange("(s p) t -> p s t", p=P))
                return xb

            w1gv = W1G.ap()
            w2gv = W2G.ap()
            x_tiles = {}
            rw_pre = []

            # ---------------- Phase B: router ----------------
            with tc.tile_pool(name="rpsum", bufs=1, space="PSUM") as rpsum, \
                 tc.tile_pool(name="rsb", bufs=6) as rsb, \
                 tc.tile_pool(name="xrt", bufs=4) as xrt, \
                 tc.tile_pool(name="lgp", bufs=2, space="PSUM") as lgp, \
                 tc.tile_pool(name="trp", bufs=3, space="PSUM") as trp, \
                 tc.tile_pool(name="upp", bufs=2, space="PSUM") as upp:

                # prefetch batch-0 router chunks under the setup latency
                xt_pre = []
                for t4 in range(TC):
                    xt = xrt.tile([P, DS, TCH], F16, tag="xrt", name="xt")
                    xq = nc.sync if t4 % 2 else nc.scalar
                    xq.dma_start(
                        xt[:],
                        xT.ap()[0].rearrange("(s p) t -> p s t", p=P)[
                            :, :, t4 * TCH:(t4 + 1) * TCH])
                    xt_pre.append(xt)

                # denominators: denom[b] = clip(sum_t mask, 1); invbc = 1/denom bcast
                mpart = rsb.tile([P, NB], F32)
                for b in range(NB):
                    nc.vector.tensor_reduce(
                        mpart[:, b:b + 1], maskT_sb[:, :, b], axis=AX.X, op=ALU.add)
                den_ps = rpsum.tile([NB, 1], F32, tag="rps")
                nc.tensor.matmul(den_ps[:], mpart[:], ones_col[:], start=True, stop=True)
                den_sb = rsb.tile([NB, 1], F32)
                nc.vector.tensor_scalar_max(den_sb[:], den_ps[:], 1.0)
                inv_sb = rsb.tile([NB, 1], F32)
                nc.vector.reciprocal(inv_sb[:], den_sb[:])
                invT_ps = rpsum.tile([1, NB], F32, tag="rps")
                nc.tensor.transpose(invT_ps[:], inv_sb[:], ident[:NB, :NB])
                invT_sb = rsb.tile([1, NB], F32)
                nc.vector.tensor_copy(invT_sb[:], invT_ps[:])
                invbc_ps = rpsum.tile([P, NB], F32, tag="rps")
                nc.tensor.matmul(invbc_ps[:], ones_row[:], invT_sb[:], start=True, stop=True)
                nc.vector.tensor_copy(invbc_sb[:], invbc_ps[:])

                NQ = TCH // P  # 4 transpose sub-chunks per 512 chunk

                def router_batch(b):
                    # maskS = mask * inv_denom for this b (free-dim broadcast)
                    maskS = rsb.tile([P, L // P], F32, tag="maskS")
                    nc.vector.tensor_tensor(
                        maskS[:], maskT_sb[:, :, b],
                        invbc_sb[:, b:b + 1].to_broadcast((P, L // P)), ALU.mult)
                    up_ps = upp.tile([E, 1], F32)
                    pend = []  # software pipeline: up-matmuls lag one chunk
                    for t4 in range(TC):
                        if b == 0:
                            xt = xt_pre[t4]
                        else:
                            xt = xrt.tile([P, DS, TCH], F16, tag="xrt",
                                          name="xt")
                            xq = nc.sync if (b * TC + t4) % 2 else nc.scalar
                            xq.dma_start(
                                xt[:],
                                xT.ap()[b].rearrange("(s p) t -> p s t", p=P)[
                                    :, :, t4 * TCH:(t4 + 1) * TCH])
                        lg_ps = lgp.tile([E, TCH], F32)
                        for dsb in range(DS):
                            nc.tensor.matmul(lg_ps[:], rwT_sb[:, dsb], xt[:, dsb],
                                             start=(dsb == 0), stop=(dsb == DS - 1))
                        lgT = rsb.tile([E, TCH], F32, tag="lgT")
                        nc.scalar.activation(lgT[:], lg_ps[:], AF.Identity, bias=rb_sb[:])
                        # 4 transposes into one psum tile [P, 4*E]
                        tr_ps = trp.tile([P, NQ * E], F32)
                        for q in range(NQ):
                            nc.tensor.matmul(
                                tr_ps[:, q * E:(q + 1) * E],
                                lgT[:, q * P:(q + 1) * P], ident[:E, :E],
                                is_transpose=True,
                                start=(q == 0), stop=(q == NQ - 1))
                        pexp = rsb.tile([P, NQ, E], F32, tag="pexp")
                        nc.scalar.activation(pexp[:], tr_ps[:], AF.Exp)
                        s4 = rsb.tile([P, NQ], F32, tag="s4")
                        nc.vector.tensor_reduce(s4[:], pexp[:], axis=AX.X, op=ALU.add)
                        sr4 = rsb.tile([P, NQ], F32, tag="sr4")
                        nc.vector.reciprocal(sr4[:], s4[:])
                        r4 = rsb.tile([P, NQ], F32, tag="r4")
                        nc.vector.tensor_tensor(
                            r4[:], sr4[:], maskS[:, t4 * NQ:(t4 + 1) * NQ], ALU.mult)
                        pend.append((pexp, r4, t4))
                        if t4 > 0:
                            pp, rr, tt = pend.pop(0)
                            for q in range(NQ):
                                nc.tensor.matmul(
                                    up_ps[:], pp[:, q], rr[:, q:q + 1],
                                    start=(tt == 0 and q == 0), stop=False)
                    pp, rr, tt = pend.pop(0)
                    for q in range(NQ):
                        nc.tensor.matmul(
                            up_ps[:], pp[:, q], rr[:, q:q + 1],
                            start=False, stop=(q == NQ - 1))
                    nc.vector.tensor_copy(up_sb[:, b:b + 1], up_ps[:])

                router_batch(0)
                # prefetch the first merge chunks and x batches while the
                # remaining routers run (queues: rw on sync/scalar behind
                # batch-0 xt loads; x on the idle gpsimd swdge queue)
                rw_pre.extend(load_rw4(w1gv, g4) for g4 in range(3))
                x_tiles[0] = load_x(0)
                x_tiles[1] = load_x(1)
                for b in range(1, NB):
                    router_batch(b)

                # broadcast up across partitions; owner-masked copy for b2
                upT_ps = rpsum.tile([NB, E], F32, tag="rps")
                nc.tensor.transpose(upT_ps[:], up_sb[:], ident[:E, :E])
                nc.vector.tensor_copy(upT_sb[:], upT_ps[:])
                nc.vector.tensor_scalar_mul(upTo_sb[:], upT_sb[:], own_sb[:])
                for b in range(NB):
                    rowu = rsb.tile([1, E], F32, tag="rowu")
                    nc.sync.dma_start(rowu[:], upT_sb[b:b + 1, :])
                    rowo = rsb.tile([1, E], F32, tag="rowo")
                    nc.sync.dma_start(rowo[:], upTo_sb[b:b + 1, :])
                    bc_ps = rpsum.tile([P, E], F32, tag="rps")
                    nc.tensor.matmul(bc_ps[:], ones_row[:], rowu[:], start=True, stop=True)
                    nc.vector.tensor_copy(up_bc[:, b], bc_ps[:])
                    bo_ps = rpsum.tile([P, E], F32, tag="rps")
                    nc.tensor.matmul(bo_ps[:], ones_row[:], rowo[:], start=True, stop=True)
                    nc.vector.tensor_copy(upo_bc[:, b], bo_ps[:])

                # block-diag stationary for the PE merge:
                # upblk[k*E+e, b*KMRG+k] = up[b, e].  DVE can't write at
                # partition offsets, so scatter with tiny SBUF->SBUF DMAs.
                uph_sb = rsb.tile([E, NB], F16)
                nc.vector.tensor_copy(uph_sb[:], up_sb[:])
                for k in range(KMRG):
                    eng = nc.scalar if k % 2 else nc.sync
                    eng.dma_start(
                        upblk[k * E:(k + 1) * E, k * NB:(k + 1) * NB],
                        uph_sb[:])

                # merged biases: mb1[b] = sum_e up[b,e] b1T[:,e]; mb2 owner-masked
                for b in range(NB):
                    nc.vector.tensor_scalar_mul(
                        mb1_sb[:, b], b1T_sb[:, :, 0], up_bc[:, b, 0:1])
                    nc.vector.tensor_scalar_mul(
                        mb2_sb[:, b], b2T_sb[:, :, 0], upo_bc[:, b, 0:1])
                    for e in range(1, E):
                        nc.vector.scalar_tensor_tensor(
                            mb1_sb[:, b], b1T_sb[:, :, e], up_bc[:, b, e:e + 1],
                            mb1_sb[:, b], ALU.mult, ALU.add)
                        nc.vector.scalar_tensor_tensor(
                            mb2_sb[:, b], b2T_sb[:, :, e], upo_bc[:, b, e:e + 1],
                            mb2_sb[:, b], ALU.mult, ALU.add)

            # ---- Phases C (merge, PE block-diag matmuls) and D (MLP) ----
            # Pools open together so SBUF regions are disjoint: no false
            # WAR deps between late merge ops and MLP tiles.
            NSB = NG // 16        # 4 superblocks of 16 row-groups
            with tc.tile_pool(name="mop", bufs=3) as mop, \
                 tc.tile_pool(name="hidp", bufs=2) as hidp, \
                 tc.tile_pool(name="wtp", bufs=3) as wtp, \
                 tc.tile_pool(name="osbp", bufs=4) as osbp, \
                 tc.tile_pool(name="mmp", bufs=2, space="PSUM") as mmp:

                def merge_w(rawv, dst, dr, pre=()):
                    """dst[b][r, c] = sum_e up[b,e] raw[e, r, c]; raw rows
                    grouped 16 at a time across PE partitions.  Column
                    halves share one PSUM bank; drains alternate ACT/DVE;
                    chunk loads and mW writes are split across queues so no
                    single 22.5 GB/s DMA engine paces the stream."""
                    for sb in range(8):
                        mos = mop.tile([P, 8, HHALF], F16, tag="mo",
                                       name="mos")
                        for gg in range(8):
                            g = sb * 8 + gg
                            if g % 4 == 0:
                                g4 = g // 4
                                rw4 = (pre[g4] if g4 < len(pre)
                                       else load_rw4(rawv, g4))
                            rw = rw4[:, g % 4]
                            ps = mmp.tile([P, TCH], F32, tag=f"ps{gg % 4}",
                                          name="psm")
                            for c in range(2):
                                nc.tensor.matmul(
                                    ps[c * 64:(c + 1) * 64, :], upblk[:],
                                    rw[:, c * HHALF:(c + 1) * HHALF],
                                    start=True, stop=True)
                            if dr[0] % 2 == 0:
                                nc.scalar.activation(mos[:, gg, :], ps[:],
                                                     AF.Identity)
                            else:
                                nc.vector.tensor_copy(mos[:, gg, :], ps[:])
                            dr[0] += 1
                        for c in range(2):
                            nc.gpsimd.dma_start(
                                dst.ap()[sb, :, :, :,
                                         c * HHALF:(c + 1) * HHALF]
                                .rearrange("gg k b h -> (k b) gg h"),
                                mos[c * 64:(c + 1) * 64])

                # ---------------- Phase D: MLP ----------------
                hid_tiles = {}

                def l1(b):
                    xb = x_tiles[b] if b in x_tiles else load_x(b)
                    hidb = hidp.tile([P, HSUB, L], F16, tag="hid", name="hidb")
                    hid_tiles[b] = hidb
                    for hb in range(HSUB):
                        w1t = wtp.tile([P, DS, P], F16, tag="w1t", name="w1t")
                        wq = nc.sync if hb % 2 else nc.scalar
                        wq.dma_start(
                            w1t[:],
                            mW1d.ap()[:, :, :, b, hb * P:(hb + 1) * P]
                            .rearrange("s gg k h -> (gg k) s h"))
                        pss = [mmp.tile([P, TCH], F32, tag=f"ps{q}",
                                        name=f"ps{q}")
                               for q in range(TC)]
                        for dsb in range(DS):
                            for q in range(TC):
                                nc.tensor.matmul(
                                    pss[q][:], w1t[:, dsb],
                                    xb[:, dsb, q * TCH:(q + 1) * TCH],
                                    start=(dsb == 0), stop=(dsb == DS - 1))
                        for q in range(TC):
                            nc.scalar.activation(
                                hidb[:, hb, q * TCH:(q + 1) * TCH], pss[q][:],
                                AF.Relu, bias=mb1_sb[:, b, hb:hb + 1])

                def l2(b):
                    hidb = hid_tiles[b]
                    for ob in range(OSUB):
                        w2t = wtp.tile([P, HSUB, P], F16, tag="w2t", name="w2t")
                        wq = nc.sync if ob % 2 else nc.scalar
                        wq.dma_start(
                            w2t[:],
                            mW2d.ap()[:, :, :, b, ob * P:(ob + 1) * P]
                            .rearrange("s gg k o -> (gg k) s o"))
                        pss = [mmp.tile([P, TCH], F32, tag=f"ps{q}",
                                        name=f"ps{q}")
                               for q in range(TC)]
                        for hs in range(HSUB):
                            for q in range(TC):
                                nc.tensor.matmul(
                                    pss[q][:], w2t[:, hs],
                                    hidb[:, hs, q * TCH:(q + 1) * TCH],
                                    start=(hs == 0), stop=(hs == HSUB - 1))
                        for q in range(TC):
                            ot = osbp.tile([P, TCH], F16, tag="ot", name="ot")
                            nc.vector.tensor_scalar_add(
                                ot[:], pss[q][:], mb2_sb[:, b, ob:ob + 1])
                            oq = nc.sync if q % 2 else nc.scalar
                            oq.dma_start(
                                outp.ap()[b, ob * P:(ob + 1) * P,
                                          q * TCH:(q + 1) * TCH], ot[:])

                drc = [0]
                merge_w(w1gv, mW1d, drc, pre=rw_pre)
                rw2_pre = [load_rw4(w2gv, g4) for g4 in range(3)]
                l1(0)
                merge_w(w2gv, mW2d, drc, pre=rw2_pre)
                l1(1)
                l2(0)
                l1(2)
                l2(1)
                l1(3)
                l2(2)
                l2(3)

            xp.release()
            rwp.release()

    nc.compile()
    return nc


def _get_nc():
    global _CACHED_NC
    if _CACHED_NC is None:
        _CACHED_NC = _build()
    return _CACHED_NC


def kernel(x, mask, router_w, router_b, W1, b1, W2, b2, _trace=False):
    x = np.asarray(x, np.float32)
    mask = np.asarray(mask, np.float32)
    router_w = np.asarray(router_w, np.float32)
    router_b = np.asarray(router_b, np.float32)
    W1 = np.asarray(W1, np.float32)
    b1 = np.asarray(b1, np.float32)
    W2 = np.asarray(W2, np.float32)
    b2 = np.asarray(b2, np.float32)

    nc = _get_nc()

    # host-side layout prep (sharding): transposes + fp16 casts
    xT_all = np.ascontiguousarray(x.transpose(0, 2, 1)).astype(np.float16)
    W1T_all = W1.transpose(0, 2, 1).astype(np.float16)    # [E, D, H]
    W2T_all = W2.transpose(0, 2, 1).astype(np.float16)    # [E, H, D]
    # small tensors pre-grouped to [partition, sub, E] so the const DMAs
    # are dense (strided 32B-line DMAs hog a queue for ~16us each)
    rwg = np.ascontiguousarray(
        router_w.T.reshape(DS, P, E).transpose(1, 0, 2)).astype(np.float16)
    rbc = np.ascontiguousarray(router_b.reshape(E, 1))
    b1T_full = np.ascontiguousarray(b1.T)                 # [H, E]
    b2g = np.ascontiguousarray(
        b2.T.reshape(OSUB, P, E).transpose(1, 0, 2))      # [P, OSUB, E]

    in_maps = []
    for c in range(8):
        g, r = c // 4, c % 4
        hs = slice(r * HS, (r + 1) * HS)
        own = np.zeros((NB, 1), np.float32)
        own[r, 0] = 1.0
        w1g = W1T_all[:, :, hs].reshape(E, D // 16, 16, HS).transpose(
            1, 2, 0, 3).reshape(D // 16, 128, HS)
        w2g = W2T_all[:, hs, :].reshape(E, HS // 16, 16, D).transpose(
            1, 2, 0, 3).reshape(HS // 16, 128, D)
        in_maps.append({
            "xT": xT_all[g * NB:(g + 1) * NB],
            "maskg": np.ascontiguousarray(
                mask[g * NB:(g + 1) * NB].T.reshape(L // P, P, NB)
                .transpose(1, 0, 2)),
            "rwg": rwg,
            "rb": rbc,
            "W1G": np.ascontiguousarray(w1g),
            "W2G": np.ascontiguousarray(w2g),
            "b1g": np.ascontiguousarray(
                b1T_full[hs].reshape(HSUB, P, E).transpose(1, 0, 2)),
            "b2g": b2g,
            "ownc": own,
        })

    res = run_bass_kernel_spmd(nc, in_maps, core_ids=list(range(8)),
                               trace=_trace)

    out = np.empty((B, L, D), np.float32)
    for g in range(2):
        acc = res.results[g * 4]["outp"].astype(np.float32)
        for r in range(1, 4):
            acc += res.results[g * 4 + r]["outp"].astype(np.float32)
        for j in range(NB):
            out[g * NB + j] = acc[j].T
    if _trace:
        return out, res
    return out


# revision 26
# speedup vs baseline: 1.1165x; 1.1165x over previous
"""SMEAR MoE layer (nn_MoELayer_SMEAR) Trainium2 Bass kernel, v2.

Problem: B=8, L=2048, D=1024, H=4096, E=8, fp32 in/out.
  logits = x @ router_w.T + router_b; probs = softmax(logits) * mask
  up = probs.sum(L) / clip(mask.sum(L), 1)            # [B, E]
  mW1 = up @ W1 ; mW2 = up @ W2 ; mb1 = up @ b1 ; mb2 = up @ b2
  out = relu(x @ mW1.T + mb1) @ mW2.T + mb2

Sharding (8 cores): dp=2 over B x tp=4 over H; host sums the 4 partial
outputs per dp-group.

v2 design (vs v1 which ran merge on PE and serialized phases):
- fp16 weight path end to end (x, W, merged W, hid, out partials).
  Numerically validated: max rel err ~6e-3 vs the 2e-2 budget (bf16 was
  1.7e-2+, too close).
- Weight merge runs on DVE + Pool(gpsimd), NOT on PE, overlapped with
  the MLP. W1 is merged in two H-half passes so L1 can start after the
  first pass; W2 merges under L1's shadow, in two D-half passes so early
  L2 output tiles unblock sooner.
- Merged weights round-trip DRAM in fp16, split into per-batch/per-half
  tensors so Tile's per-tensor DRAM dep tracking gives fine-grained
  readiness.
- MLP keeps each stationary tile for 4 back-to-back matmuls into 4 PSUM
  banks (LDWEIGHTS amortized; 8 banks double-buffer across groups).
- PE order L1(0) L1(1) L2(0) L1(2) L2(1) L1(3) L2(2) L2(3) so W2-merge
  latency hides while keeping only 2 hid buffers resident.
"""

import numpy as np

import concourse.bass as bass
import concourse.bacc as bacc
import concourse.mybir as mybir
import concourse.tile as tile
from concourse.bass_utils import run_bass_kernel_spmd
from concourse.masks import make_identity

P = 128
B, L, D, H, E = 8, 2048, 1024, 4096, 8
NB = 4          # batches per core
HS = H // 4     # h-shard width per core
DS = D // P     # 8 d-subtiles
HSUB = HS // P  # 8 h-subtiles in shard
OSUB = D // P   # 8 output subtiles
TCH = 512       # moving-dim chunk for matmuls
TC = L // TCH   # 4 chunks per batch
HHALF = HS // 2  # merge half-pass width

F32 = mybir.dt.float32
F16 = mybir.dt.float16
AF = mybir.ActivationFunctionType
ALU = mybir.AluOpType
AX = mybir.AxisListType

KMRG = 16           # weight rows merged per matmul (16 rows x 8 experts = 128)
NG = D * HS // (KMRG * HS)  # 64 row-groups per weight matrix

_CACHED_NC = None


def _build():
    nc = bacc.Bacc("TRN2", target_bir_lowering=False, debug=False)

    xT = nc.dram_tensor("xT", [NB, D, L], F16, kind="ExternalInput")
    maskg = nc.dram_tensor("maskg", [P, L // P, NB], F32, kind="ExternalInput")
    rwg = nc.dram_tensor("rwg", [P, DS, E], F16, kind="ExternalInput")
    rb = nc.dram_tensor("rb", [E, 1], F32, kind="ExternalInput")
    # raw weights pre-grouped on host: [g, k*E+e, c] = WT[e, g*KMRG+k, c]
    W1G = nc.dram_tensor("W1G", [D // KMRG, P, HS], F16, kind="ExternalInput")
    W2G = nc.dram_tensor("W2G", [HS // KMRG, P, D], F16, kind="ExternalInput")
    b1g = nc.dram_tensor("b1g", [P, HSUB, E], F32, kind="ExternalInput")
    b2g = nc.dram_tensor("b2g", [P, OSUB, E], F32, kind="ExternalInput")
    ownc = nc.dram_tensor("ownc", [NB, 1], F32, kind="ExternalInput")
    outp = nc.dram_tensor("outp", [NB, D, L], F16, kind="ExternalOutput")

    # merged weights in (superblock, group, batch, row, col) layout so
    # merge writes are 2 big DMAs per superblock and stationary-tile
    # reads are one DMA each
    mW1d = nc.dram_tensor("mW1d", [8, 8, KMRG, NB, HS], F16)
    mW2d = nc.dram_tensor("mW2d", [8, 8, KMRG, NB, D], F16)

    with tile.TileContext(nc) as tc:
        with tc.tile_pool(name="const", bufs=1) as const:
            ident = const.tile([P, P], F32)
            make_identity(nc, ident)
            ones_col = const.tile([P, 1], F32)
            nc.gpsimd.memset(ones_col[:], 1.0)
            ones_row = const.tile([1, P], F32)
            nc.gpsimd.memset(ones_row[:], 1.0)

            rwT_sb = const.tile([P, DS, E], F16)
            nc.sync.dma_start(rwT_sb[:], rwg.ap())
            rb_sb = const.tile([E, 1], F32)
            nc.sync.dma_start(rb_sb[:], rb.ap())
            maskT_sb = const.tile([P, L // P, NB], F32)
            nc.sync.dma_start(maskT_sb[:], maskg.ap())
            b1T_sb = const.tile([P, HSUB, E], F32)
            nc.scalar.dma_start(b1T_sb[:], b1g.ap())
            b2T_sb = const.tile([P, OSUB, E], F32)
            nc.scalar.dma_start(b2T_sb[:], b2g.ap())
            own_sb = const.tile([NB, 1], F32)
            nc.sync.dma_start(own_sb[:], ownc.ap())

            up_sb = const.tile([E, NB], F32)
            upT_sb = const.tile([NB, E], F32)
            upTo_sb = const.tile([NB, E], F32)
            up_bc = const.tile([P, NB, E], F32)
            upo_bc = const.tile([P, NB, E], F32)
            mb1_sb = const.tile([P, NB, HSUB], F32)
            mb2_sb = const.tile([P, NB, OSUB], F32)
            invbc_sb = const.tile([P, NB], F32)
            # block-diag merge stationary: upblk[k*E+e, k*NB+b] = up[b, e]
            upblk = const.tile([P, NB * KMRG], F16)
            nc.gpsimd.memset(upblk[:], 0.0)

            # rwp/xp open before the router so the W1 merge chunks and
            # first x batches stream during phase B.
            rwp = tc.alloc_tile_pool(name="rwp", bufs=4)
            xp = tc.alloc_tile_pool(name="xp", bufs=2)

            def load_rw4(rawv, g4):
                # one 1MB DMA covering 4 row-groups; big transfers fan out
                # across DMA engines, small ones don't
                rw = rwp.tile([P, 4, HS], F16, tag="rw", name="rw")
                eng = nc.sync if g4 % 2 else nc.scalar
                eng.dma_start(
                    rw[:], rawv[4 * g4:4 * g4 + 4].rearrange("g p h -> p g h"))
                return rw

            def load_x(b):
                xb = xp.tile([P, DS, L], F16, tag="x", name="xb")
                nc.gpsimd.dma_start(
                    xb[:], xT.ap()[b].rearrange("(s p) t -> p s t", p=P))
                return xb

            w1gv = W1G.ap()
            w2gv = W2G.ap()
            x_tiles = {}
            rw_pre = []

            # ---------------- Phase B: router ----------------
            with tc.tile_pool(name="rpsum", bufs=1, space="PSUM") as rpsum, \
                 tc.tile_pool(name="rsb", bufs=6) as rsb, \
                 tc.tile_pool(name="xrt", bufs=4) as xrt, \
                 tc.tile_pool(name="lgp", bufs=2, space="PSUM") as lgp, \
                 tc.tile_pool(name="trp", bufs=3, space="PSUM") as trp, \
                 tc.tile_pool(name="upp", bufs=2, space="PSUM") as upp:

                # prefetch batch-0 router chunks under the setup latency
                xt_pre = []
                for t4 in range(TC):
                    xt = xrt.tile([P, DS, TCH], F16, tag="xrt", name="xt")
                    xq = nc.sync if t4 % 2 else nc.scalar
                    xq.dma_start(
                        xt[:],
                        xT.ap()[0].rearrange("(s p) t -> p s t", p=P)[
                            :, :, t4 * TCH:(t4 + 1) * TCH])
                    xt_pre.append(xt)

                # denominators: denom[b] = clip(sum_t mask, 1); invbc = 1/denom bcast
                mpart = rsb.tile([P, NB], F32)
                for b in range(NB):
                    nc.vector.tensor_reduce(
                        mpart[:, b:b + 1], maskT_sb[:, :, b], axis=AX.X, op=ALU.add)
                den_ps = rpsum.tile([NB, 1], F32, tag="rps")
                nc.tensor.matmul(den_ps[:], mpart[:], ones_col[:], start=True, stop=True)
                den_sb = rsb.tile([NB, 1], F32)
                nc.vector.tensor_scalar_max(den_sb[:], den_ps[:], 1.0)
                inv_sb = rsb.tile([NB, 1], F32)
                nc.vector.reciprocal(inv_sb[:], den_sb[:])
                invT_ps = rpsum.tile([1, NB], F32, tag="rps")
                nc.tensor.transpose(invT_ps[:], inv_sb[:], ident[:NB, :NB])
                invT_sb = rsb.tile([1, NB], F32)
                nc.vector.tensor_copy(invT_sb[:], invT_ps[:])
                invbc_ps = rpsum.tile([P, NB], F32, tag="rps")
                nc.tensor.matmul(invbc_ps[:], ones_row[:], invT_sb[:], start=True, stop=True)
                nc.vector.tensor_copy(invbc_sb[:], invbc_ps[:])

                NQ = TCH // P  # 4 transpose sub-chunks per 512 chunk

                def router_batch(b):
                    # maskS = mask * inv_denom for this b (free-dim broadcast)
                    maskS = rsb.tile([P, L // P], F32, tag="maskS")
                    nc.vector.tensor_tensor(
                        maskS[:], maskT_sb[:, :, b],
                        invbc_sb[:, b:b + 1].to_broadcast((P, L // P)), ALU.mult)
                    up_ps = upp.tile([E, 1], F32)
                    pend = []  # software pipeline: up-matmuls lag one chunk
                    for t4 in range(TC):
                        if b == 0:
                            xt = xt_pre[t4]
                        else:
                            xt = xrt.tile([P, DS, TCH], F16, tag="xrt",
                                          name="xt")
                            xq = nc.sync if (b * TC + t4) % 2 else nc.scalar
                            xq.dma_start(
                                xt[:],
                                xT.ap()[b].rearrange("(s p) t -> p s t", p=P)[
                                    :, :, t4 * TCH:(t4 + 1) * TCH])
                        lg_ps = lgp.tile([E, TCH], F32)
                        for dsb in range(DS):
                            nc.tensor.matmul(lg_ps[:], rwT_sb[:, dsb], xt[:, dsb],
                                             start=(dsb == 0), stop=(dsb == DS - 1))
                        lgT = rsb.tile([E, TCH], F32, tag="lgT")
                        nc.scalar.activation(lgT[:], lg_ps[:], AF.Identity, bias=rb_sb[:])
                        # 4 transposes into one psum tile [P, 4*E]
                        tr_ps = trp.tile([P, NQ * E], F32)
                        for q in range(NQ):
                            nc.tensor.matmul(
                                tr_ps[:, q * E:(q + 1) * E],
                                lgT[:, q * P:(q + 1) * P], ident[:E, :E],
                                is_transpose=True,
                                start=(q == 0), stop=(q == NQ - 1))
                        pexp = rsb.tile([P, NQ, E], F32, tag="pexp")
                        nc.scalar.activation(pexp[:], tr_ps[:], AF.Exp)
                        s4 = rsb.tile([P, NQ], F32, tag="s4")
                        nc.vector.tensor_reduce(s4[:], pexp[:], axis=AX.X, op=ALU.add)
                        sr4 = rsb.tile([P, NQ], F32, tag="sr4")
                        nc.vector.reciprocal(sr4[:], s4[:])
                        r4 = rsb.tile([P, NQ], F32, tag="r4")
                        nc.vector.tensor_tensor(
                            r4[:], sr4[:], maskS[:, t4 * NQ:(t4 + 1) * NQ], ALU.mult)
                        pend.append((pexp, r4, t4))
                        if t4 > 0:
                            pp, rr, tt = pend.pop(0)
                            for q in range(NQ):
                                nc.tensor.matmul(
                                    up_ps[:], pp[:, q], rr[:, q:q + 1],
                                    start=(tt == 0 and q == 0), stop=False)
                    pp, rr, tt = pend.pop(0)
                    for q in range(NQ):
                        nc.tensor.matmul(
                            up_ps[:], pp[:, q], rr[:, q:q + 1],
                            start=False, stop=(q == NQ - 1))
                    nc.vector.tensor_copy(up_sb[:, b:b + 1], up_ps[:])

                # stagger prefetches so they don't steal HBM bandwidth
                # from the router's own chunk stream (the critical path)
                router_batch(0)
                router_batch(1)
                rw_pre.extend(load_rw4(w1gv, g4) for g4 in range(3))
                router_batch(2)
                x_tiles[0] = load_x(0)
                router_batch(3)
                x_tiles[1] = load_x(1)

                # broadcast up across partitions; owner-masked copy for b2
                upT_ps = rpsum.tile([NB, E], F32, tag="rps")
                nc.tensor.transpose(upT_ps[:], up_sb[:], ident[:E, :E])
                nc.vector.tensor_copy(upT_sb[:], upT_ps[:])
                nc.vector.tensor_scalar_mul(upTo_sb[:], upT_sb[:], own_sb[:])
                for b in range(NB):
                    rowu = rsb.tile([1, E], F32, tag="rowu")
                    nc.sync.dma_start(rowu[:], upT_sb[b:b + 1, :])
                    rowo = rsb.tile([1, E], F32, tag="rowo")
                    nc.sync.dma_start(rowo[:], upTo_sb[b:b + 1, :])
                    bc_ps = rpsum.tile([P, E], F32, tag="rps")
                    nc.tensor.matmul(bc_ps[:], ones_row[:], rowu[:], start=True, stop=True)
                    nc.vector.tensor_copy(up_bc[:, b], bc_ps[:])
                    bo_ps = rpsum.tile([P, E], F32, tag="rps")
                    nc.tensor.matmul(bo_ps[:], ones_row[:], rowo[:], start=True, stop=True)
                    nc.vector.tensor_copy(upo_bc[:, b], bo_ps[:])

                # block-diag stationary for the PE merge:
                # upblk[k*E+e, b*KMRG+k] = up[b, e].  DVE can't write at
                # partition offsets, so scatter with tiny SBUF->SBUF DMAs.
                uph_sb = rsb.tile([E, NB], F16)
                nc.vector.tensor_copy(uph_sb[:], up_sb[:])
                for k in range(KMRG):
                    eng = nc.scalar if k % 2 else nc.sync
                    eng.dma_start(
                        upblk[k * E:(k + 1) * E, k * NB:(k + 1) * NB],
                        uph_sb[:])

                # merged biases: mb1[b] = sum_e up[b,e] b1T[:,e]; mb2 owner-masked
                for b in range(NB):
                    nc.vector.tensor_scalar_mul(
                        mb1_sb[:, b], b1T_sb[:, :, 0], up_bc[:, b, 0:1])
                    nc.vector.tensor_scalar_mul(
                        mb2_sb[:, b], b2T_sb[:, :, 0], upo_bc[:, b, 0:1])
                    for e in range(1, E):
                        nc.vector.scalar_tensor_tensor(
                            mb1_sb[:, b], b1T_sb[:, :, e], up_bc[:, b, e:e + 1],
                            mb1_sb[:, b], ALU.mult, ALU.add)
                        nc.vector.scalar_tensor_tensor(
                            mb2_sb[:, b], b2T_sb[:, :, e], upo_bc[:, b, e:e + 1],
                            mb2_sb[:, b], ALU.mult, ALU.add)

            # ---- Phases C (merge, PE block-diag matmuls) and D (MLP) ----
            # Pools open together so SBUF regions are disjoint: no false
            # WAR deps between late merge ops and MLP tiles.
            NSB = NG // 16        # 4 superblocks of 16 row-groups
            with tc.tile_pool(name="mop", bufs=3) as mop, \
                 tc.tile_pool(name="hidp", bufs=2) as hidp, \
                 tc.tile_pool(name="wtp", bufs=3) as wtp, \
                 tc.tile_pool(name="osbp", bufs=4) as osbp, \
                 tc.tile_pool(name="mmp", bufs=2, space="PSUM") as mmp:

                def merge_w(rawv, dst, dr, pre=()):
                    """dst[b][r, c] = sum_e up[b,e] raw[e, r, c]; raw rows
                    grouped 16 at a time across PE partitions.  Column
                    halves share one PSUM bank; drains alternate ACT/DVE;
                    chunk loads and mW writes are split across queues so no
                    single 22.5 GB/s DMA engine paces the stream."""
                    for sb in range(8):
                        mos = mop.tile([P, 8, HHALF], F16, tag="mo",
                                       name="mos")
                        for gg in range(8):
                            g = sb * 8 + gg
                            if g % 4 == 0:
                                g4 = g // 4
                                rw4 = (pre[g4] if g4 < len(pre)
                                       else load_rw4(rawv, g4))
                            rw = rw4[:, g % 4]
                            ps = mmp.tile([P, TCH], F32, tag=f"ps{gg % 4}",
                                          name="psm")
                            for c in range(2):
                                nc.tensor.matmul(
                                    ps[c * 64:(c + 1) * 64, :], upblk[:],
                                    rw[:, c * HHALF:(c + 1) * HHALF],
                                    start=True, stop=True)
                            if dr[0] % 2 == 0:
                                nc.scalar.activation(mos[:, gg, :], ps[:],
                                                     AF.Identity)
                            else:
                                nc.vector.tensor_copy(mos[:, gg, :], ps[:])
                            dr[0] += 1
                        for c in range(2):
                            nc.gpsimd.dma_start(
                                dst.ap()[sb, :, :, :,
                                         c * HHALF:(c + 1) * HHALF]
                                .rearrange("gg k b h -> (k b) gg h"),
                                mos[c * 64:(c + 1) * 64])

                # ---------------- Phase D: MLP ----------------
                hid_tiles = {}

                def l1(b):
                    xb = x_tiles[b] if b in x_tiles else load_x(b)
                    hidb = hidp.tile([P, HSUB, L], F16, tag="hid", name="hidb")
                    hid_tiles[b] = hidb
                    for hb in range(HSUB):
                        w1t = wtp.tile([P, DS, P], F16, tag="w1t", name="w1t")
                        wq = nc.sync if hb % 2 else nc.scalar
                        wq.dma_start(
                            w1t[:],
                            mW1d.ap()[:, :, :, b, hb * P:(hb + 1) * P]
                            .rearrange("s gg k h -> (gg k) s h"))
                        pss = [mmp.tile([P, TCH], F32, tag=f"ps{q}",
                                        name=f"ps{q}")
                               for q in range(TC)]
                        for dsb in range(DS):
                            for q in range(TC):
                                nc.tensor.matmul(
                                    pss[q][:], w1t[:, dsb],
                                    xb[:, dsb, q * TCH:(q + 1) * TCH],
                                    start=(dsb == 0), stop=(dsb == DS - 1))
                        for q in range(TC):
                            nc.scalar.activation(
                                hidb[:, hb, q * TCH:(q + 1) * TCH], pss[q][:],
                                AF.Relu, bias=mb1_sb[:, b, hb:hb + 1])

                def l2(b):
                    hidb = hid_tiles[b]
                    for ob in range(OSUB):
                        w2t = wtp.tile([P, HSUB, P], F16, tag="w2t", name="w2t")
                        wq = nc.sync if ob % 2 else nc.scalar
                        wq.dma_start(
                            w2t[:],
                            mW2d.ap()[:, :, :, b, ob * P:(ob + 1) * P]
                            .rearrange("s gg k o -> (gg k) s o"))
                        pss = [mmp.tile([P, TCH], F32, tag=f"ps{q}",
                                        name=f"ps{q}")
                               for q in range(TC)]
                        for hs in range(HSUB):
                            for q in range(TC):
                                nc.tensor.matmul(
                                    pss[q][:], w2t[:, hs],
                                    hidb[:, hs, q * TCH:(q + 1) * TCH],
                                    start=(hs == 0), stop=(hs == HSUB - 1))
                        for q in range(TC):
                            ot = osbp.tile([P, TCH], F16, tag="ot", name="ot")
                            nc.vector.tensor_scalar_add(
                                ot[:], pss[q][:], mb2_sb[:, b, ob:ob + 1])
                            oq = nc.sync if q % 2 else nc.scalar
                            oq.dma_start(
                                outp.ap()[b, ob * P:(ob + 1) * P,
                                          q * TCH:(q + 1) * TCH], ot[:])

                drc = [0]
                merge_w(w1gv, mW1d, drc, pre=rw_pre)
                rw2_pre = [load_rw4(w2gv, g4) for g4 in range(3)]
                l1(0)
                merge_w(w2gv, mW2d, drc, pre=rw2_pre)
                l1(1)
                l2(0)
                l1(2)
                l2(1)
                l1(3)
                l2(2)
                l2(3)

            xp.release()
            rwp.release()

    nc.compile()
    return nc


def _get_nc():
    global _CACHED_NC
    if _CACHED_NC is None:
        _CACHED_NC = _build()
    return _CACHED_NC


def kernel(x, mask, router_w, router_b, W1, b1, W2, b2, _trace=False):
    x = np.asarray(x, np.float32)
    mask = np.asarray(mask, np.float32)
    router_w = np.asarray(router_w, np.float32)
    router_b = np.asarray(router_b, np.float32)
    W1 = np.asarray(W1, np.float32)
    b1 = np.asarray(b1, np.float32)
    W2 = np.asarray(W2, np.float32)
    b2 = np.asarray(b2, np.float32)

    nc = _get_nc()

    # host-side layout prep (sharding): transposes + fp16 casts
    xT_all = np.ascontiguousarray(x.transpose(0, 2, 1)).astype(np.float16)
    W1T_all = W1.transpose(0, 2, 1).astype(np.float16)    # [E, D, H]
    W2T_all = W2.transpose(0, 2, 1).astype(np.float16)    # [E, H, D]
    # small tensors pre-grouped to [partition, sub, E] so the const DMAs
    # are dense (strided 32B-line DMAs hog a queue for ~16us each)
    rwg = np.ascontiguousarray(
        router_w.T.reshape(DS, P, E).transpose(1, 0, 2)).astype(np.float16)
    rbc = np.ascontiguousarray(router_b.reshape(E, 1))
    b1T_full = np.ascontiguousarray(b1.T)                 # [H, E]
    b2g = np.ascontiguousarray(
        b2.T.reshape(OSUB, P, E).transpose(1, 0, 2))      # [P, OSUB, E]

    in_maps = []
    for c in range(8):
        g, r = c // 4, c % 4
        hs = slice(r * HS, (r + 1) * HS)
        own = np.zeros((NB, 1), np.float32)
        own[r, 0] = 1.0
        w1g = W1T_all[:, :, hs].reshape(E, D // 16, 16, HS).transpose(
            1, 2, 0, 3).reshape(D // 16, 128, HS)
        w2g = W2T_all[:, hs, :].reshape(E, HS // 16, 16, D).transpose(
            1, 2, 0, 3).reshape(HS // 16, 128, D)
        in_maps.append({
            "xT": xT_all[g * NB:(g + 1) * NB],
            "maskg": np.ascontiguousarray(
                mask[g * NB:(g + 1) * NB].T.reshape(L // P, P, NB)
                .transpose(1, 0, 2)),
            "rwg": rwg,
            "rb": rbc,
            "W1G": np.ascontiguousarray(w1g),
            "W2G": np.ascontiguousarray(w2g),
            "b1g": np.ascontiguousarray(
                b1T_full[hs].reshape(HSUB, P, E).transpose(1, 0, 2)),
            "b2g": b2g,
            "ownc": own,
        })

    res = run_bass_kernel_spmd(nc, in_maps, core_ids=list(range(8)),
                               trace=_trace)

    out = np.empty((B, L, D), np.float32)
    for g in range(2):
        acc = res.results[g * 4]["outp"].astype(np.float32)
        for r in range(1, 4):
            acc += res.results[g * 4 + r]["outp"].astype(np.float32)
        for j in range(NB):
            out[g * NB + j] = acc[j].T
    if _trace:
        return out, res
    return out


# revision 27
# speedup vs baseline: 1.1980x; 1.0730x over previous
"""SMEAR MoE layer (nn_MoELayer_SMEAR) Trainium2 Bass kernel, v2.

Problem: B=8, L=2048, D=1024, H=4096, E=8, fp32 in/out.
  logits = x @ router_w.T + router_b; probs = softmax(logits) * mask
  up = probs.sum(L) / clip(mask.sum(L), 1)            # [B, E]
  mW1 = up @ W1 ; mW2 = up @ W2 ; mb1 = up @ b1 ; mb2 = up @ b2
  out = relu(x @ mW1.T + mb1) @ mW2.T + mb2

Sharding (8 cores): dp=2 over B x tp=4 over H; host sums the 4 partial
outputs per dp-group.

v2 design (vs v1 which ran merge on PE and serialized phases):
- fp16 weight path end to end (x, W, merged W, hid, out partials).
  Numerically validated: max rel err ~6e-3 vs the 2e-2 budget (bf16 was
  1.7e-2+, too close).
- Weight merge runs on DVE + Pool(gpsimd), NOT on PE, overlapped with
  the MLP. W1 is merged in two H-half passes so L1 can start after the
  first pass; W2 merges under L1's shadow, in two D-half passes so early
  L2 output tiles unblock sooner.
- Merged weights round-trip DRAM in fp16, split into per-batch/per-half
  tensors so Tile's per-tensor DRAM dep tracking gives fine-grained
  readiness.
- MLP keeps each stationary tile for 4 back-to-back matmuls into 4 PSUM
  banks (LDWEIGHTS amortized; 8 banks double-buffer across groups).
- PE order L1(0) L1(1) L2(0) L1(2) L2(1) L1(3) L2(2) L2(3) so W2-merge
  latency hides while keeping only 2 hid buffers resident.
"""

import numpy as np

import concourse.bass as bass
import concourse.bacc as bacc
import concourse.mybir as mybir
import concourse.tile as tile
from concourse.bass_utils import run_bass_kernel_spmd
from concourse.masks import make_identity

P = 128
B, L, D, H, E = 8, 2048, 1024, 4096, 8
NB = 4          # batches per core
HS = H // 4     # h-shard width per core
DS = D // P     # 8 d-subtiles
HSUB = HS // P  # 8 h-subtiles in shard
OSUB = D // P   # 8 output subtiles
TCH = 512       # moving-dim chunk for matmuls
TC = L // TCH   # 4 chunks per batch
HHALF = HS // 2  # merge half-pass width

F32 = mybir.dt.float32
F16 = mybir.dt.float16
AF = mybir.ActivationFunctionType
ALU = mybir.AluOpType
AX = mybir.AxisListType

KMRG = 16           # weight rows merged per matmul (16 rows x 8 experts = 128)
NG = D * HS // (KMRG * HS)  # 64 row-groups per weight matrix

_CACHED_NC = None


def _build():
    nc = bacc.Bacc("TRN2", target_bir_lowering=False, debug=False)

    xT = nc.dram_tensor("xT", [NB, D, L], F16, kind="ExternalInput")
    maskg = nc.dram_tensor("maskg", [P, L // P, NB], F32, kind="ExternalInput")
    rwg = nc.dram_tensor("rwg", [P, DS, E], F16, kind="ExternalInput")
    rb = nc.dram_tensor("rb", [E, 1], F32, kind="ExternalInput")
    # raw weights pre-grouped on host: [g, k*E+e, c] = WT[e, g*KMRG+k, c]
    W1G = nc.dram_tensor("W1G", [D // KMRG, P, HS], F16, kind="ExternalInput")
    W2G = nc.dram_tensor("W2G", [HS // KMRG, P, D], F16, kind="ExternalInput")
    b1g = nc.dram_tensor("b1g", [P, HSUB, E], F32, kind="ExternalInput")
    b2g = nc.dram_tensor("b2g", [P, OSUB, E], F32, kind="ExternalInput")
    ownc = nc.dram_tensor("ownc", [NB, 1], F32, kind="ExternalInput")
    outp = nc.dram_tensor("outp", [NB, D, L], F16, kind="ExternalOutput")

    # merged weights in (superblock, group, batch, row, col) layout so
    # merge writes are 2 big DMAs per superblock and stationary-tile
    # reads are one DMA each
    mW1d = nc.dram_tensor("mW1d", [8, 8, KMRG, NB, HS], F16)
    mW2d = nc.dram_tensor("mW2d", [8, 8, KMRG, NB, D], F16)

    with tile.TileContext(nc) as tc:
        with tc.tile_pool(name="const", bufs=1) as const:
            ident = const.tile([P, P], F32)
            make_identity(nc, ident)
            ones_col = const.tile([P, 1], F32)
            nc.gpsimd.memset(ones_col[:], 1.0)
            ones_row = const.tile([1, P], F32)
            nc.gpsimd.memset(ones_row[:], 1.0)

            rwT_sb = const.tile([P, DS, E], F16)
            nc.sync.dma_start(rwT_sb[:], rwg.ap())
            rb_sb = const.tile([E, 1], F32)
            nc.sync.dma_start(rb_sb[:], rb.ap())
            maskT_sb = const.tile([P, L // P, NB], F32)
            nc.sync.dma_start(maskT_sb[:], maskg.ap())
            b1T_sb = const.tile([P, HSUB, E], F32)
            nc.scalar.dma_start(b1T_sb[:], b1g.ap())
            b2T_sb = const.tile([P, OSUB, E], F32)
            nc.scalar.dma_start(b2T_sb[:], b2g.ap())
            own_sb = const.tile([NB, 1], F32)
            nc.sync.dma_start(own_sb[:], ownc.ap())

            up_sb = const.tile([E, NB], F32)
            upT_sb = const.tile([NB, E], F32)
            upTo_sb = const.tile([NB, E], F32)
            up_bc = const.tile([P, NB, E], F32)
            upo_bc = const.tile([P, NB, E], F32)
            mb1_sb = const.tile([P, NB, HSUB], F32)
            mb2_sb = const.tile([P, NB, OSUB], F32)
            invbc_sb = const.tile([P, NB], F32)
            # block-diag merge stationary: upblk[k*E+e, k*NB+b] = up[b, e]
            upblk = const.tile([P, NB * KMRG], F16)
            nc.gpsimd.memset(upblk[:], 0.0)

            # rwp/xp open before the router so the W1 merge chunks and
            # first x batches stream during phase B.
            rwp = tc.alloc_tile_pool(name="rwp", bufs=4)
            xp = tc.alloc_tile_pool(name="xp", bufs=2)

            def load_rw4(rawv, g4):
                # one 1MB DMA covering 4 row-groups; big transfers fan out
                # across DMA engines, small ones don't
                rw = rwp.tile([P, 4, HS], F16, tag="rw", name="rw")
                eng = nc.sync if g4 % 2 else nc.scalar
                eng.dma_start(
                    rw[:], rawv[4 * g4:4 * g4 + 4].rearrange("g p h -> p g h"))
                return rw

            def load_x(b, hot=False):
                xb = xp.tile([P, DS, L], F16, tag="x", name="xb")
                xv = xT.ap()[b].rearrange("(s p) t -> p s t", p=P)
                for t4 in range(TC):
                    if hot:
                        eng = nc.sync if (b + t4) % 2 else nc.scalar
                    else:
                        eng = nc.gpsimd
                    eng.dma_start(xb[:, :, t4 * TCH:(t4 + 1) * TCH],
                                  xv[:, :, t4 * TCH:(t4 + 1) * TCH])
                return xb

            w1gv = W1G.ap()
            w2gv = W2G.ap()
            x_tiles = {}
            rw_pre = []

            # ---------------- Phase B: router ----------------
            with tc.tile_pool(name="rpsum", bufs=1, space="PSUM") as rpsum, \
                 tc.tile_pool(name="rsb", bufs=6) as rsb, \
                 tc.tile_pool(name="xrt", bufs=4) as xrt, \
                 tc.tile_pool(name="lgp", bufs=2, space="PSUM") as lgp, \
                 tc.tile_pool(name="trp", bufs=3, space="PSUM") as trp, \
                 tc.tile_pool(name="upp", bufs=2, space="PSUM") as upp:

                # batches 0/1: load full x up front (quartered so the
                # router consumes incrementally); L1 reuses the same tiles,
                # saving 8MB of re-read on the critical path
                x_tiles[0] = load_x(0, hot=True)
                x_tiles[1] = load_x(1, hot=True)

                # denominators: denom[b] = clip(sum_t mask, 1); invbc = 1/denom bcast
                mpart = rsb.tile([P, NB], F32)
                for b in range(NB):
                    nc.vector.tensor_reduce(
                        mpart[:, b:b + 1], maskT_sb[:, :, b], axis=AX.X, op=ALU.add)
                den_ps = rpsum.tile([NB, 1], F32, tag="rps")
                nc.tensor.matmul(den_ps[:], mpart[:], ones_col[:], start=True, stop=True)
                den_sb = rsb.tile([NB, 1], F32)
                nc.vector.tensor_scalar_max(den_sb[:], den_ps[:], 1.0)
                inv_sb = rsb.tile([NB, 1], F32)
                nc.vector.reciprocal(inv_sb[:], den_sb[:])
                invT_ps = rpsum.tile([1, NB], F32, tag="rps")
                nc.tensor.transpose(invT_ps[:], inv_sb[:], ident[:NB, :NB])
                invT_sb = rsb.tile([1, NB], F32)
                nc.vector.tensor_copy(invT_sb[:], invT_ps[:])
                invbc_ps = rpsum.tile([P, NB], F32, tag="rps")
                nc.tensor.matmul(invbc_ps[:], ones_row[:], invT_sb[:], start=True, stop=True)
                nc.vector.tensor_copy(invbc_sb[:], invbc_ps[:])

                NQ = TCH // P  # 4 transpose sub-chunks per 512 chunk

                def router_batch(b):
                    # maskS = mask * inv_denom for this b (free-dim broadcast)
                    maskS = rsb.tile([P, L // P], F32, tag="maskS")
                    nc.vector.tensor_tensor(
                        maskS[:], maskT_sb[:, :, b],
                        invbc_sb[:, b:b + 1].to_broadcast((P, L // P)), ALU.mult)
                    up_ps = upp.tile([E, 1], F32)
                    pend = []  # software pipeline: up-matmuls lag one chunk
                    for t4 in range(TC):
                        if b < 2:
                            xt = x_tiles[b][:, :, t4 * TCH:(t4 + 1) * TCH]
                        else:
                            xtt = xrt.tile([P, DS, TCH], F16, tag="xrt",
                                           name="xtt")
                            xq = nc.sync if (b * TC + t4) % 2 else nc.scalar
                            xq.dma_start(
                                xtt[:],
                                xT.ap()[b].rearrange("(s p) t -> p s t", p=P)[
                                    :, :, t4 * TCH:(t4 + 1) * TCH])
                            xt = xtt[:]
                        lg_ps = lgp.tile([E, TCH], F32)
                        for dsb in range(DS):
                            nc.tensor.matmul(lg_ps[:], rwT_sb[:, dsb], xt[:, dsb],
                                             start=(dsb == 0), stop=(dsb == DS - 1))
                        lgT = rsb.tile([E, TCH], F32, tag="lgT")
                        nc.scalar.activation(lgT[:], lg_ps[:], AF.Identity, bias=rb_sb[:])
                        # 4 transposes into one psum tile [P, 4*E]
                        tr_ps = trp.tile([P, NQ * E], F32)
                        for q in range(NQ):
                            nc.tensor.matmul(
                                tr_ps[:, q * E:(q + 1) * E],
                                lgT[:, q * P:(q + 1) * P], ident[:E, :E],
                                is_transpose=True,
                                start=(q == 0), stop=(q == NQ - 1))
                        pexp = rsb.tile([P, NQ, E], F32, tag="pexp")
                        nc.scalar.activation(pexp[:], tr_ps[:], AF.Exp)
                        s4 = rsb.tile([P, NQ], F32, tag="s4")
                        nc.vector.tensor_reduce(s4[:], pexp[:], axis=AX.X, op=ALU.add)
                        sr4 = rsb.tile([P, NQ], F32, tag="sr4")
                        nc.vector.reciprocal(sr4[:], s4[:])
                        r4 = rsb.tile([P, NQ], F32, tag="r4")
                        nc.vector.tensor_tensor(
                            r4[:], sr4[:], maskS[:, t4 * NQ:(t4 + 1) * NQ], ALU.mult)
                        pend.append((pexp, r4, t4))
                        if t4 > 0:
                            pp, rr, tt = pend.pop(0)
                            for q in range(NQ):
                                nc.tensor.matmul(
                                    up_ps[:], pp[:, q], rr[:, q:q + 1],
                                    start=(tt == 0 and q == 0), stop=False)
                    pp, rr, tt = pend.pop(0)
                    for q in range(NQ):
                        nc.tensor.matmul(
                            up_ps[:], pp[:, q], rr[:, q:q + 1],
                            start=False, stop=(q == NQ - 1))
                    nc.vector.tensor_copy(up_sb[:, b:b + 1], up_ps[:])

                # stagger the merge-chunk prefetch so it doesn't steal
                # HBM bandwidth from the router's own stream
                router_batch(0)
                router_batch(1)
                rw_pre.extend(load_rw4(w1gv, g4) for g4 in range(3))
                router_batch(2)
                router_batch(3)

                # broadcast up across partitions; owner-masked copy for b2
                upT_ps = rpsum.tile([NB, E], F32, tag="rps")
                nc.tensor.transpose(upT_ps[:], up_sb[:], ident[:E, :E])
                nc.vector.tensor_copy(upT_sb[:], upT_ps[:])
                nc.vector.tensor_scalar_mul(upTo_sb[:], upT_sb[:], own_sb[:])
                for b in range(NB):
                    rowu = rsb.tile([1, E], F32, tag="rowu")
                    nc.sync.dma_start(rowu[:], upT_sb[b:b + 1, :])
                    rowo = rsb.tile([1, E], F32, tag="rowo")
                    nc.sync.dma_start(rowo[:], upTo_sb[b:b + 1, :])
                    bc_ps = rpsum.tile([P, E], F32, tag="rps")
                    nc.tensor.matmul(bc_ps[:], ones_row[:], rowu[:], start=True, stop=True)
                    nc.vector.tensor_copy(up_bc[:, b], bc_ps[:])
                    bo_ps = rpsum.tile([P, E], F32, tag="rps")
                    nc.tensor.matmul(bo_ps[:], ones_row[:], rowo[:], start=True, stop=True)
                    nc.vector.tensor_copy(upo_bc[:, b], bo_ps[:])

                # block-diag stationary for the PE merge:
                # upblk[k*E+e, b*KMRG+k] = up[b, e].  DVE can't write at
                # partition offsets, so scatter with tiny SBUF->SBUF DMAs.
                uph_sb = rsb.tile([E, NB], F16)
                nc.vector.tensor_copy(uph_sb[:], up_sb[:])
                for k in range(KMRG):
                    eng = nc.scalar if k % 2 else nc.sync
                    eng.dma_start(
                        upblk[k * E:(k + 1) * E, k * NB:(k + 1) * NB],
                        uph_sb[:])

                # merged biases: mb1[b] = sum_e up[b,e] b1T[:,e]; mb2 owner-masked
                for b in range(NB):
                    nc.vector.tensor_scalar_mul(
                        mb1_sb[:, b], b1T_sb[:, :, 0], up_bc[:, b, 0:1])
                    nc.vector.tensor_scalar_mul(
                        mb2_sb[:, b], b2T_sb[:, :, 0], upo_bc[:, b, 0:1])
                    for e in range(1, E):
                        nc.vector.scalar_tensor_tensor(
                            mb1_sb[:, b], b1T_sb[:, :, e], up_bc[:, b, e:e + 1],
                            mb1_sb[:, b], ALU.mult, ALU.add)
                        nc.vector.scalar_tensor_tensor(
                            mb2_sb[:, b], b2T_sb[:, :, e], upo_bc[:, b, e:e + 1],
                            mb2_sb[:, b], ALU.mult, ALU.add)

            # ---- Phases C (merge, PE block-diag matmuls) and D (MLP) ----
            # Pools open together so SBUF regions are disjoint: no false
            # WAR deps between late merge ops and MLP tiles.
            NSB = NG // 16        # 4 superblocks of 16 row-groups
            with tc.tile_pool(name="mop", bufs=3) as mop, \
                 tc.tile_pool(name="hidp", bufs=2) as hidp, \
                 tc.tile_pool(name="wtp", bufs=3) as wtp, \
                 tc.tile_pool(name="osbp", bufs=4) as osbp, \
                 tc.tile_pool(name="mmp", bufs=2, space="PSUM") as mmp:

                def merge_w(rawv, dst, dr, pre=()):
                    """dst[b][r, c] = sum_e up[b,e] raw[e, r, c]; raw rows
                    grouped 16 at a time across PE partitions.  Column
                    halves share one PSUM bank; drains alternate ACT/DVE;
                    chunk loads and mW writes are split across queues so no
                    single 22.5 GB/s DMA engine paces the stream."""
                    for sb in range(8):
                        mos = mop.tile([P, 8, HHALF], F16, tag="mo",
                                       name="mos")
                        for gg in range(8):
                            g = sb * 8 + gg
                            if g % 4 == 0:
                                g4 = g // 4
                                rw4 = (pre[g4] if g4 < len(pre)
                                       else load_rw4(rawv, g4))
                            rw = rw4[:, g % 4]
                            ps = mmp.tile([P, TCH], F32, tag=f"ps{gg % 4}",
                                          name="psm")
                            for c in range(2):
                                nc.tensor.matmul(
                                    ps[c * 64:(c + 1) * 64, :], upblk[:],
                                    rw[:, c * HHALF:(c + 1) * HHALF],
                                    start=True, stop=True)
                            if dr[0] % 2 == 0:
                                nc.scalar.activation(mos[:, gg, :], ps[:],
                                                     AF.Identity)
                            else:
                                nc.vector.tensor_copy(mos[:, gg, :], ps[:])
                            dr[0] += 1
                        for c in range(2):
                            nc.gpsimd.dma_start(
                                dst.ap()[sb, :, :, :,
                                         c * HHALF:(c + 1) * HHALF]
                                .rearrange("gg k b h -> (k b) gg h"),
                                mos[c * 64:(c + 1) * 64])

                # ---------------- Phase D: MLP ----------------
                hid_tiles = {}

                def l1(b):
                    xb = x_tiles[b] if b in x_tiles else load_x(b)
                    hidb = hidp.tile([P, HSUB, L], F16, tag="hid", name="hidb")
                    hid_tiles[b] = hidb
                    for hb in range(HSUB):
                        w1t = wtp.tile([P, DS, P], F16, tag="w1t", name="w1t")
                        wq = nc.sync if hb % 2 else nc.scalar
                        wq.dma_start(
                            w1t[:],
                            mW1d.ap()[:, :, :, b, hb * P:(hb + 1) * P]
                            .rearrange("s gg k h -> (gg k) s h"))
                        pss = [mmp.tile([P, TCH], F32, tag=f"ps{q}",
                                        name=f"ps{q}")
                               for q in range(TC)]
                        for dsb in range(DS):
                            for q in range(TC):
                                nc.tensor.matmul(
                                    pss[q][:], w1t[:, dsb],
                                    xb[:, dsb, q * TCH:(q + 1) * TCH],
                                    start=(dsb == 0), stop=(dsb == DS - 1))
                        for q in range(TC):
                            nc.scalar.activation(
                                hidb[:, hb, q * TCH:(q + 1) * TCH], pss[q][:],
                                AF.Relu, bias=mb1_sb[:, b, hb:hb + 1])

                def l2(b):
                    hidb = hid_tiles[b]
                    for ob in range(OSUB):
                        w2t = wtp.tile([P, HSUB, P], F16, tag="w2t", name="w2t")
                        wq = nc.sync if ob % 2 else nc.scalar
                        wq.dma_start(
                            w2t[:],
                            mW2d.ap()[:, :, :, b, ob * P:(ob + 1) * P]
                            .rearrange("s gg k o -> (gg k) s o"))
                        pss = [mmp.tile([P, TCH], F32, tag=f"ps{q}",
                                        name=f"ps{q}")
                               for q in range(TC)]
                        for hs in range(HSUB):
                            for q in range(TC):
                                nc.tensor.matmul(
                                    pss[q][:], w2t[:, hs],
                                    hidb[:, hs, q * TCH:(q + 1) * TCH],
                                    start=(hs == 0), stop=(hs == HSUB - 1))
                        for q in range(TC):
                            ot = osbp.tile([P, TCH], F16, tag="ot", name="ot")
                            nc.vector.tensor_scalar_add(
                                ot[:], pss[q][:], mb2_sb[:, b, ob:ob + 1])
                            oq = nc.sync if q % 2 else nc.scalar
                            oq.dma_start(
                                outp.ap()[b, ob * P:(ob + 1) * P,
                                          q * TCH:(q + 1) * TCH], ot[:])

                drc = [0]
                merge_w(w1gv, mW1d, drc, pre=rw_pre)
                rw2_pre = [load_rw4(w2gv, g4) for g4 in range(3)]
                l1(0)
                merge_w(w2gv, mW2d, drc, pre=rw2_pre)
                l1(1)
                l2(0)
                l1(2)
                l2(1)
                l1(3)
                l2(2)
                l2(3)

            xp.release()
            rwp.release()

    nc.compile()
    return nc


def _get_nc():
    global _CACHED_NC
    if _CACHED_NC is None:
        _CACHED_NC = _build()
    return _CACHED_NC


def kernel(x, mask, router_w, router_b, W1, b1, W2, b2, _trace=False):
    x = np.asarray(x, np.float32)
    mask = np.asarray(mask, np.float32)
    router_w = np.asarray(router_w, np.float32)
    router_b = np.asarray(router_b, np.float32)
    W1 = np.asarray(W1, np.float32)
    b1 = np.asarray(b1, np.float32)
    W2 = np.asarray(W2, np.float32)
    b2 = np.asarray(b2, np.float32)

    nc = _get_nc()

    # host-side layout prep (sharding): transposes + fp16 casts
    xT_all = np.ascontiguousarray(x.transpose(0, 2, 1)).astype(np.float16)
    W1T_all = W1.transpose(0, 2, 1).astype(np.float16)    # [E, D, H]
    W2T_all = W2.transpose(0, 2, 1).astype(np.float16)    # [E, H, D]
    # small tensors pre-grouped to [partition, sub, E] so the const DMAs
    # are dense (strided 32B-line DMAs hog a queue for ~16us each)
    rwg = np.ascontiguousarray(
        router_w.T.reshape(DS, P, E).transpose(1, 0, 2)).astype(np.float16)
    rbc = np.ascontiguousarray(router_b.reshape(E, 1))
    b1T_full = np.ascontiguousarray(b1.T)                 # [H, E]
    b2g = np.ascontiguousarray(
        b2.T.reshape(OSUB, P, E).transpose(1, 0, 2))      # [P, OSUB, E]

    in_maps = []
    for c in range(8):
        g, r = c // 4, c % 4
        hs = slice(r * HS, (r + 1) * HS)
        own = np.zeros((NB, 1), np.float32)
        own[r, 0] = 1.0
        w1g = W1T_all[:, :, hs].reshape(E, D // 16, 16, HS).transpose(
            1, 2, 0, 3).reshape(D // 16, 128, HS)
        w2g = W2T_all[:, hs, :].reshape(E, HS // 16, 16, D).transpose(
            1, 2, 0, 3).reshape(HS // 16, 128, D)
        in_maps.append({
            "xT": xT_all[g * NB:(g + 1) * NB],
            "maskg": np.ascontiguousarray(
                mask[g * NB:(g + 1) * NB].T.reshape(L // P, P, NB)
                .transpose(1, 0, 2)),
            "rwg": rwg,
            "rb": rbc,
            "W1G": np.ascontiguousarray(w1g),
            "W2G": np.ascontiguousarray(w2g),
            "b1g": np.ascontiguousarray(
                b1T_full[hs].reshape(HSUB, P, E).transpose(1, 0, 2)),
            "b2g": b2g,
            "ownc": own,
        })

    res = run_bass_kernel_spmd(nc, in_maps, core_ids=list(range(8)),
                               trace=_trace)

    out = np.empty((B, L, D), np.float32)
    for g in range(2):
        acc = res.results[g * 4]["outp"].astype(np.float32)
        for r in range(1, 4):
            acc += res.results[g * 4 + r]["outp"].astype(np.float32)
        for j in range(NB):
            out[g * NB + j] = acc[j].T
    if _trace:
        return out, res
    return out
